# revision 52
# baseline (speedup 1.0000x reference)
"""AttentivePooling Trainium2 kernel (8 NeuronCores, data-parallel over batch).

Algorithm (mathematically exact refactoring of the reference):
  scores[b,h,q,s] = <hs[b,s,:], Wtilde[:,(h,q)]>  with
      Wtilde = Wk^T @ Qhat,  Qhat[g,(h,q)] = scale * q[q, g] * [g in head h block]
      (the k-projection bias drops out via softmax shift invariance)
  p = exp(scores)            (scores are O(0.05); no max subtraction needed)
  den[(h,q)] = sum_s p;  num[(h,q),e] = sum_s p * hs[b,s,e]
  attn_weights[b,q,s] = (1/H) sum_h p[s,(h,q)] / den[(h,q)]
  nbar[h,e] = (1/Q) sum_q num[(h,q),e]/den[(h,q)]
  cbar[h*D+d] = (nbar @ Wv^T)[h, h*D+d] + bv[h*D+d]
  pooled[b,:] = out_w @ cbar + out_b

Each core handles one batch element: a single streaming pass over its
16 MiB hidden_states shard (memory-bound), with scores computed via
PE-transposed tiles and num/den accumulated in PSUM across the pass.
"""

import sys

if "/opt/trn_rl_repo" not in sys.path:
    sys.path.insert(0, "/opt/trn_rl_repo")

import numpy as np
import ml_dtypes

import concourse.bass as bass
import concourse.tile as tile
from concourse import mybir
from concourse.bass_utils import run_bass_kernel_spmd

BF16 = ml_dtypes.bfloat16
F32 = mybir.dt.float32
BF = mybir.dt.bfloat16

B, S, E, Q, H = 8, 4096, 1024, 4, 4
D = E // H          # 256
QH = H * Q          # 16
CH = E // 128       # 8 chunks of the feature dim
TILES = S // 128    # 32 s-tiles per core
SCALE = 1.0 / np.sqrt(D)
N_CORES = 8


def _patch_tile_drain():
    """The walrus build here caps sync-waits per instruction (1 for most
    opcodes, 2 for EventSemaphore), but the Tile scheduler freely attaches
    several. Split the excess onto preceding same-engine nops at commit
    time (commit order == final program order), and likewise for the exit
    drain's accumulated waits."""
    from concourse.tile import ScopedClock, TileContext

    orig_commit = TileContext._commit_instruction

    def _commit_instruction(self, inst, lazy_reg_writes=True):
        si = getattr(inst, "sync_info", None)
        cap = 2 if isinstance(inst, mybir.InstEventSemaphore) else 1
        engine = getattr(inst, "engine", None)
        if si is not None and si.on_wait and len(si.on_wait) > cap and engine:
            waits = list(si.on_wait)
            for w in waits[:-cap]:
                nop = mybir.InstNoOp(
                    name=self.nc.get_next_instruction_name(),
                    engine=engine,
                    bass_nofuse=True,
                    sync_info=mybir.SyncInfo(on_wait=[w], on_update=[]),
                    ins=[],
                    outs=[],
                )
                orig_commit(self, nop, lazy_reg_writes)
            inst.sync_info = mybir.SyncInfo(
                on_wait=waits[-cap:], on_update=list(si.on_update)
            )
        return orig_commit(self, inst, lazy_reg_writes)

    TileContext._commit_instruction = _commit_instruction

    def _drain_and_barrier(self, tick_clock, wait_clock):
        nc = self.nc
        drain_inst = nc.sync.drain()
        wait_clock.add_sem_waits(
            drain_inst.ins, ScopedClock({None: tick_clock.global_clock})
        )
        waits = list(drain_inst.ins.sync_info.on_wait)
        if len(waits) > 1:
            drain_inst.ins.sync_info.on_wait = [waits[0]]
            for w in waits[1:]:
                n = nc.sync.nop(nofuse=True)
                n.ins.sync_info = mybir.SyncInfo(on_wait=[w], on_update=[])

        nc.all_engine_barrier()
        assert self.sems is not None
        popped = nc._tile_sem_poison_stack.pop()
        assert popped is self._sem_poison
        nc.clear_and_free_semaphores(list(self.sems.allocated().values()))
        nc.all_engine_barrier()

    TileContext._drain_and_barrier = _drain_and_barrier


_patch_tile_drain()


def _patch_walrus_ldw_opt():
    """bir_verify_and_optimise hardcodes --enable-ldw-opt=false; enable it
    so LDWEIGHTS can use the fast-weight-load path / pull-ahead."""
    import os
    import concourse.bass_utils as bu

    # walrus rejects Tile's explicit InstLdweights with the opt enabled;
    # kept only as an experiment flag.
    if os.environ.get("BASS_LDW_OPT", "0") != "1":
        return
    orig_run = bu.run_command

    def run_command(argv, **kw):
        argv = [
            a.replace("--enable-ldw-opt=false", "--enable-ldw-opt=true")
            if isinstance(a, str)
            else a
            for a in argv
        ]
        return orig_run(argv, **kw)

    bu.run_command = run_command


_patch_walrus_ldw_opt()


def _build_nc() -> bass.Bass:
    nc = bass.Bass()

    hs = nc.declare_dram_parameter("hs", [S, E], F32, isOutput=False)
    qT = nc.declare_dram_parameter("qT", [128, CH * Q], BF, isOutput=False)
    wq = nc.declare_dram_parameter("wq", [E, E], BF, isOutput=False)   # Wq^T
    wk = nc.declare_dram_parameter("wk", [E, E], BF, isOutput=False)   # Wk natural
    wv = nc.declare_dram_parameter("wv", [E, E], BF, isOutput=False)   # Wv^T
    ow = nc.declare_dram_parameter("ow", [E, E], BF, isOutput=False)   # out_w^T
    bq = nc.declare_dram_parameter("bq", [1, E], BF, isOutput=False)
    bv = nc.declare_dram_parameter("bv", [1, E], BF, isOutput=False)
    ob = nc.declare_dram_parameter("ob", [1, E], BF, isOutput=False)
    idb = nc.declare_dram_parameter("idb", [128, 128], BF, isOutput=False)
    ones14 = nc.declare_dram_parameter("ones14", [1, 4], BF, isOutput=False)
    one11 = nc.declare_dram_parameter("one11", [1, 1], BF, isOutput=False)
    maskG = nc.declare_dram_parameter("maskG", [QH, H], BF, isOutput=False)
    maskA = nc.declare_dram_parameter("maskA", [QH, Q], BF, isOutput=False)

    pooled_out = nc.declare_dram_parameter("pooled", [1, E], F32, isOutput=True)
    attnw_out = nc.declare_dram_parameter("attn_w", [Q, S], F32, isOutput=True)

    Exp = mybir.ActivationFunctionType.Exp

    with tile.TileContext(nc) as tc:
        with (
            tc.tile_pool(name="consts", bufs=1) as consts,
            tc.tile_pool(name="weights", bufs=1) as weights,
            tc.tile_pool(name="small", bufs=1) as small,
        ):
            # ---- constants ----
            # tiny SWDGE warmer: absorbs the ~13us Q7/SWDGE spin-up so the
            # first real hs supertile DMA streams immediately
            warm_sb = consts.tile([128, 16], F32)
            nc.gpsimd.dma_start(warm_sb[:], hs[0:128, 0:16])

            idb_sb = consts.tile([128, 128], BF)
            nc.sync.dma_start(idb_sb[:], idb[:])
            ones14_sb = consts.tile([1, 4], BF)
            nc.sync.dma_start(ones14_sb[:], ones14[:])
            one11_sb = consts.tile([1, 1], BF)
            nc.sync.dma_start(one11_sb[:], one11[:])
            maskG_sb = consts.tile([QH, H], BF)
            nc.sync.dma_start(maskG_sb[:], maskG[:])
            maskA_sb = consts.tile([QH, Q], BF)
            nc.sync.dma_start(maskA_sb[:], maskA[:])
            bq_sb = consts.tile([1, E], BF)
            nc.sync.dma_start(bq_sb[:], bq[:])
            bv_sb = consts.tile([1, E], BF)
            nc.sync.dma_start(bv_sb[:], bv[:])
            ob_sb = consts.tile([1, E], BF)
            nc.sync.dma_start(ob_sb[:], ob[:])
            # qT arrives host-packed as [128, CH*Q] (single clean DMA)
            qT_sb = consts.tile([128, CH * Q], BF)
            nc.sync.dma_start(qT_sb[:], qT[:])

            # prefetch the very first hs tile over the HWDGE sync ring
            # (sub-us spin-up) so PE transposes start before the SWDGE
            # stream produces its first bytes (~13us spin-up)
            pre0 = small.tile([128, E], F32)
            nc.sync.dma_start(pre0[:], hs[0:128, :])

            # ---- prologue weights (needed before the scores pass) ----
            # split across both HWDGE rings; the SWDGE queue is reserved for
            # the hs stream so it can start immediately
            wq_sb = weights.tile([128, CH * E], BF)
            wk_sb = weights.tile([128, CH * E], BF)
            for dst, src in ((wq_sb, wq), (wk_sb, wk)):
                nc.scalar.dma_start(
                    dst[:].rearrange("p (c f) -> p c f", c=CH),
                    src[:].rearrange("(c p) f -> p c f", p=128),
                )

            # ---- prologue outputs (built inside the main pool scope) ----
            wt_sb = small.tile([128, CH * QH], BF)    # Wtilde chunks
            qhat_sb = small.tile([128, CH * QH], BF)  # Qhat chunks
            q_sb = small.tile([Q, E], BF)
            wtT_sb = small.tile([QH, E], BF)

            # ---- main streaming pass over hidden_states ----
            # SWDGE cast-DMA loads 4-tile supertiles f32->bf16 directly.
            # Per supertile g (512 rows of hs):
            #   transpose the 32 [128,128] blocks on PE into T (feature-major)
            #   scores2[16, 512] = sum_c wt_c^T @ T_c   (cheap 16-col ldweights)
            #   p2[:, g*512:+512] = exp(scores2), den partial via accum_out
            #   per 128-tile: p_tile[128,16] = transpose(p2 slice); num += p^T hs
            aw_sb = small.tile([Q, S], F32)
            p2_sb = small.tile([QH, S], BF)
            num_sb = small.tile([QH, E], BF)
            den8 = small.tile([QH, NSUP := TILES // 4], F32)
            den_sb = small.tile([QH, 1], F32)
            SUP = 4          # tiles per supertile DMA

            with (
                tc.tile_pool(name="stage", bufs=6) as stage,
                tc.tile_pool(name="tpool", bufs=4) as tpool,
                tc.tile_pool(name="ppool", bufs=6) as ppool,
                tc.tile_pool(name="psT", bufs=2, space="PSUM") as psT,
                tc.tile_pool(name="pscore", bufs=2, space="PSUM") as pscore,
                tc.tile_pool(name="psp", bufs=2, space="PSUM") as psp,
                tc.tile_pool(name="psnd", bufs=1, space="PSUM") as psnd,
            ):
                nd_ps = psnd.tile([QH, E], F32)

                def load_and_transpose(g):
                    # supertile 0 comes over the SP HWDGE ring as f32 (+DVE
                    # cast): HWDGE spins up in <1us while the SWDGE path,
                    # which carries the rest of the stream, takes ~13us to
                    # produce its first bytes.
                    r0 = g * SUP * 128
                    hs_b = stage.tile([128, SUP * E], BF, name="hs_b")
                    # per-tile DMAs: transposes gate on single 512KB tiles
                    for k in range(SUP):
                        if g == 0 and k == 0:
                            # tile 0 was prefetched as f32 on the sync ring
                            nc.vector.tensor_copy(hs_b[:, 0:E], pre0[:])
                            continue
                        nc.gpsimd.dma_start(
                            hs_b[:, k * E : (k + 1) * E],
                            hs[r0 + k * 128 : r0 + (k + 1) * 128, :],
                        )
                    # transposed supertile, laid out [128e, (c, 512s)]
                    T_sb = tpool.tile([128, CH, SUP * 128], BF, name="T_sb")
                    for k in range(SUP):
                        hv = hs_b[:, k * E : (k + 1) * E]
                        for half in range(2):
                            T_ps = psT.tile([128, 512], F32, name="T_ps")
                            for cc in range(4):
                                c = half * 4 + cc
                                nc.tensor.matmul(
                                    T_ps[:, cc * 128 : (cc + 1) * 128],
                                    lhsT=hv[:, c * 128 : (c + 1) * 128],
                                    rhs=idb_sb[:],
                                    start=True,
                                    stop=True,
                                )
                            eng = (
                                nc.scalar.copy
                                if (2 * k + half) % 2 == 0
                                else nc.vector.tensor_copy
                            )
                            eng(
                                T_sb[
                                    :, half * 4 : (half + 1) * 4, k * 128 : (k + 1) * 128
                                ],
                                T_ps[:].rearrange("p (c s) -> p c s", c=4),
                            )
                    return hs_b, T_sb

                def score_and_accumulate(g, hs_b, T_sb):
                    sc_ps = pscore.tile([QH, SUP * 128], F32, name="sc_ps")
                    for c in range(CH):
                        nc.tensor.matmul(
                            sc_ps[:],
                            lhsT=wt_sb[:, c * QH : (c + 1) * QH],
                            rhs=T_sb[:, c, :],
                            start=(c == 0),
                            stop=(c == CH - 1),
                        )
                    p2_sl = p2_sb[:, g * SUP * 128 : (g + 1) * SUP * 128]
                    nc.scalar.activation(
                        p2_sl, sc_ps[:], Exp, accum_out=den8[:, g : g + 1]
                    )

                    for k in range(SUP):
                        i = g * SUP + k
                        pt_ps = psp.tile([128, QH], F32, name="pt_ps")
                        nc.tensor.matmul(
                            pt_ps[:],
                            lhsT=p2_sb[:, i * 128 : (i + 1) * 128],
                            rhs=idb_sb[0:QH, 0:QH],
                            start=True,
                            stop=True,
                        )
                        p_t = ppool.tile([128, QH], BF, name="p_t")
                        nc.vector.tensor_copy(p_t[:], pt_ps[:])
                        for j in range(2):
                            nc.tensor.matmul(
                                nd_ps[0:QH, j * 512 : (j + 1) * 512],
                                lhsT=p_t[:],
                                rhs=hs_b[:, k * E + j * 512 : k * E + (j + 1) * 512],
                                start=(i == 0),
                                stop=(i == TILES - 1),
                            )

                def build_wtilde():
                    # q = queries@Wq^T + bq  (psum slots borrowed from the
                    # main-loop pools via tags so banks stay within budget)
                    q_h = [
                        pscore.tile([Q, 512], F32, name="q_h", tag="sc_ps")
                        for _ in range(2)
                    ]
                    for j in range(2):
                        for c in range(CH):
                            nc.tensor.matmul(
                                q_h[j][:],
                                lhsT=qT_sb[:, c * Q : (c + 1) * Q],
                                rhs=wq_sb[:, c * E + j * 512 : c * E + (j + 1) * 512],
                                start=(c == 0),
                                stop=False,
                            )
                        nc.tensor.matmul(
                            q_h[j][:],
                            lhsT=ones14_sb[:],
                            rhs=bq_sb[0:1, j * 512 : (j + 1) * 512],
                            start=False,
                            stop=True,
                        )
                        nc.scalar.copy(q_sb[:, j * 512 : (j + 1) * 512], q_h[j][:])

                    qt_ps = psT.tile([128, CH * Q], F32, name="qt_ps", tag="T_ps")
                    for r in range(CH):
                        nc.tensor.matmul(
                            qt_ps[:, r * Q : (r + 1) * Q],
                            lhsT=q_sb[:, r * 128 : (r + 1) * 128],
                            rhs=idb_sb[0:Q, 0:Q],
                            start=True,
                            stop=True,
                        )
                    nc.vector.memset(qhat_sb[:], 0.0)
                    for r in range(CH):
                        h = r // 2
                        nc.scalar.mul(
                            qhat_sb[:, r * QH + h * Q : r * QH + (h + 1) * Q],
                            qt_ps[:, r * Q : (r + 1) * Q],
                            SCALE,
                        )

                    # WtildeT[qh, e] = sum_g qhat[g, qh] * wk[g, e], then 8
                    # small transposes into the [e-major] wt chunks
                    for j in range(2):
                        wtT_ps = pscore.tile(
                            [QH, 512], F32, name="wtT_ps", tag="sc_ps"
                        )
                        for r in range(CH):
                            nc.tensor.matmul(
                                wtT_ps[:],
                                lhsT=qhat_sb[:, r * QH : (r + 1) * QH],
                                rhs=wk_sb[:, r * E + j * 512 : r * E + (j + 1) * 512],
                                start=(r == 0),
                                stop=(r == CH - 1),
                            )
                        nc.scalar.copy(
                            wtT_sb[:, j * 512 : (j + 1) * 512], wtT_ps[:]
                        )
                    for c in range(CH):
                        wt_ps = psp.tile([128, QH], F32, name="wt_ps", tag="pt_ps")
                        nc.tensor.matmul(
                            wt_ps[:],
                            lhsT=wtT_sb[:, c * 128 : (c + 1) * 128],
                            rhs=idb_sb[0:QH, 0:QH],
                            start=True,
                            stop=True,
                        )
                        nc.scalar.copy(wt_sb[:, c * QH : (c + 1) * QH], wt_ps[:])

                # software-pipelined; the first two supertiles' transposes are
                # traced BEFORE the prologue so the in-order PE stream starts
                # on work that only needs the (early) hs DMA, not the weights
                g0 = load_and_transpose(0)
                g1 = load_and_transpose(1)
                build_wtilde()
                prev = g1
                score_and_accumulate(0, *g0)
                for g in range(2, NSUP):
                    cur = load_and_transpose(g)
                    score_and_accumulate(g - 1, *prev)
                    prev = cur
                score_and_accumulate(NSUP - 1, *prev)

                nc.scalar.copy(num_sb[:], nd_ps[0:QH, :])
                nc.vector.reduce_sum(den_sb[:], den8[:], axis=mybir.AxisListType.X)

            # ---- epilogue weights: traced late so they fill DMA gaps ----
            wv_sb = weights.tile([128, CH * E], BF)
            ow_sb = weights.tile([128, CH * E], BF)
            nc.scalar.dma_start(
                wv_sb[:].rearrange("p (c f) -> p c f", c=CH),
                wv[:].rearrange("(c p) f -> p c f", p=128),
            )
            nc.scalar.dma_start(
                ow_sb[:].rearrange("p (c f) -> p c f", c=CH),
                ow[:].rearrange("(c p) f -> p c f", p=128),
            )

            # ---- epilogue ----
            with (
                tc.tile_pool(name="awp", bufs=2, space="PSUM") as aw_pool,
            ):
                inv_den = small.tile([QH, 1], F32)
                nc.vector.reciprocal(inv_den[:], den_sb[:])
                invq = small.tile([QH, 1], F32)
                nc.vector.tensor_scalar_mul(invq[:], inv_den[:], 1.0 / Q)
                G_sb = small.tile([QH, H], BF)
                nc.vector.tensor_scalar_mul(G_sb[:], maskG_sb[:], invq[:])
                cA_sb = small.tile([QH, Q], BF)
                nc.vector.tensor_scalar_mul(cA_sb[:], maskA_sb[:], invq[:])

                for g in range(8):
                    awp = aw_pool.tile([Q, 512], F32, name="awp")
                    nc.tensor.matmul(
                        awp[:],
                        lhsT=cA_sb[:],
                        rhs=p2_sb[:, g * 512 : (g + 1) * 512],
                        start=True,
                        stop=True,
                    )
                    nc.scalar.copy(aw_sb[:, g * 512 : (g + 1) * 512], awp[:])

            with (
                tc.tile_pool(name="nbt", bufs=1, space="PSUM") as nbt_pool,
                tc.tile_pool(name="vbp", bufs=1, space="PSUM") as vb_pool,
                tc.tile_pool(name="cbt", bufs=1, space="PSUM") as cbt_pool,
                tc.tile_pool(name="plp", bufs=1, space="PSUM") as pl_pool,
            ):
                nbt_ps = nbt_pool.tile([128, CH * H], F32)
                for c in range(CH):
                    nc.tensor.matmul(
                        nbt_ps[:, c * H : (c + 1) * H],
                        lhsT=num_sb[:, c * 128 : (c + 1) * 128],
                        rhs=G_sb[:],
                        start=True,
                        stop=True,
                    )
                nbt_sb = small.tile([128, CH * H], BF)
                nc.scalar.copy(nbt_sb[:], nbt_ps[:])

                # cbar[0, h*D+d] = sum_e nbarT[e,h] * WvT[e, h*D+d] + bv[h*D+d]
                # computed per-head directly into partition 0 (engines cannot
                # read PSUM at unaligned base partitions).
                cb_ps = vb_pool.tile([1, E], F32)
                for h in range(H):
                    for c in range(CH):
                        nc.tensor.matmul(
                            cb_ps[0:1, h * D : (h + 1) * D],
                            lhsT=nbt_sb[:, c * H + h : c * H + h + 1],
                            rhs=wv_sb[:, c * E + h * D : c * E + (h + 1) * D],
                            start=(c == 0),
                            stop=False,
                        )
                    nc.tensor.matmul(
                        cb_ps[0:1, h * D : (h + 1) * D],
                        lhsT=one11_sb[:],
                        rhs=bv_sb[0:1, h * D : (h + 1) * D],
                        start=False,
                        stop=True,
                    )
                cbar_sb = small.tile([1, E], BF)
                nc.scalar.copy(cbar_sb[:], cb_ps[:])

                cbt_ps = cbt_pool.tile([128, CH], F32)
                for c in range(CH):
                    nc.tensor.matmul(
                        cbt_ps[:, c : c + 1],
                        lhsT=cbar_sb[0:1, c * 128 : (c + 1) * 128],
                        rhs=one11_sb[:],
                        start=True,
                        stop=True,
                    )
                cbt_sb = small.tile([128, CH], BF)
                nc.scalar.copy(cbt_sb[:], cbt_ps[:])

                pl_ps = pl_pool.tile([1, E], F32)
                for j in range(2):
                    for c in range(CH):
                        nc.tensor.matmul(
                            pl_ps[0:1, j * 512 : (j + 1) * 512],
                            lhsT=cbt_sb[:, c : c + 1],
                            rhs=ow_sb[:, c * E + j * 512 : c * E + (j + 1) * 512],
                            start=(c == 0),
                            stop=False,
                        )
                    nc.tensor.matmul(
                        pl_ps[0:1, j * 512 : (j + 1) * 512],
                        lhsT=one11_sb[:],
                        rhs=ob_sb[0:1, j * 512 : (j + 1) * 512],
                        start=False,
                        stop=True,
                    )
                pooled_sb = small.tile([1, E], F32)
                nc.scalar.copy(pooled_sb[:], pl_ps[:])

            nc.sync.dma_start(attnw_out[:], aw_sb[:])
            nc.sync.dma_start(pooled_out[:], pooled_sb[:])

    return nc


_NC_CACHE = None


def _get_nc():
    global _NC_CACHE
    if _NC_CACHE is None:
        _NC_CACHE = _build_nc()
    return _NC_CACHE


def _host_prep(hidden_states, queries, in_proj_w, in_proj_b, out_w):
    f32 = np.float32
    hidden_states = np.asarray(hidden_states, dtype=f32)
    queries = np.asarray(queries, dtype=f32)
    in_proj_w = np.asarray(in_proj_w, dtype=f32)
    in_proj_b = np.asarray(in_proj_b, dtype=f32)
    out_w = np.asarray(out_w, dtype=f32)

    maskG = np.zeros((QH, H), dtype=BF16)
    maskA = np.zeros((QH, Q), dtype=BF16)
    for h in range(H):
        for q in range(Q):
            maskG[h * Q + q, h] = 1
            maskA[h * Q + q, q] = 1

    # qT packed to the SBUF layout [128, (chunk, q)]
    qT_packed = np.ascontiguousarray(
        queries[0].T.reshape(CH, 128, Q).transpose(1, 0, 2).reshape(128, CH * Q)
    ).astype(BF16)

    shared = {
        "qT": qT_packed,
        "wq": np.ascontiguousarray(in_proj_w[0:E].T).astype(BF16),
        "wk": np.ascontiguousarray(in_proj_w[E : 2 * E]).astype(BF16),
        "wv": np.ascontiguousarray(in_proj_w[2 * E : 3 * E].T).astype(BF16),
        "ow": np.ascontiguousarray(out_w.T).astype(BF16),
        "bq": in_proj_b[0:E].reshape(1, E).astype(BF16),
        "bv": in_proj_b[2 * E : 3 * E].reshape(1, E).astype(BF16),
        "ob": None,  # filled by caller (out_b)
        "idb": np.eye(128, dtype=BF16),
        "ones14": np.ones((1, 4), dtype=BF16),
        "one11": np.ones((1, 1), dtype=BF16),
        "maskG": maskG,
        "maskA": maskA,
    }
    return hidden_states, shared


def kernel(hidden_states, queries, in_proj_w, in_proj_b, out_w, out_b, _trace=False):
    hidden_states, shared = _host_prep(
        hidden_states, queries, in_proj_w, in_proj_b, out_w
    )
    shared["ob"] = np.asarray(out_b, dtype=np.float32).reshape(1, E).astype(BF16)

    in_maps = []
    for b in range(N_CORES):
        m = dict(shared)
        m["hs"] = np.ascontiguousarray(hidden_states[b])
        in_maps.append(m)

    nc = _get_nc()
    res = run_bass_kernel_spmd(nc, in_maps, core_ids=list(range(N_CORES)), trace=_trace)

    pooled = np.stack([res.results[b]["pooled"][0] for b in range(N_CORES)])
    attn_w = np.stack([res.results[b]["attn_w"] for b in range(N_CORES)])
    if _trace:
        return (pooled, attn_w), res
    return pooled, attn_w


# revision 54
# speedup vs baseline: 1.0929x; 1.0929x over previous
"""AttentivePooling Trainium2 kernel (8 NeuronCores, data-parallel over batch).

Algorithm (mathematically exact refactoring of the reference):
  scores[b,h,q,s] = <hs[b,s,:], Wtilde[:,(h,q)]>  with
      Wtilde = Wk^T @ Qhat,  Qhat[g,(h,q)] = scale * q[q, g] * [g in head h block]
      (the k-projection bias drops out via softmax shift invariance)
  p = exp(scores)            (scores are O(0.05); no max subtraction needed)
  den[(h,q)] = sum_s p;  num[(h,q),e] = sum_s p * hs[b,s,e]
  attn_weights[b,q,s] = (1/H) sum_h p[s,(h,q)] / den[(h,q)]
  nbar[h,e] = (1/Q) sum_q num[(h,q),e]/den[(h,q)]
  cbar[h*D+d] = (nbar @ Wv^T)[h, h*D+d] + bv[h*D+d]
  pooled[b,:] = out_w @ cbar + out_b

Each core handles one batch element: a single streaming pass over its
16 MiB hidden_states shard (memory-bound), with scores computed via
PE-transposed tiles and num/den accumulated in PSUM across the pass.
"""

import sys

if "/opt/trn_rl_repo" not in sys.path:
    sys.path.insert(0, "/opt/trn_rl_repo")

import numpy as np
import ml_dtypes

import concourse.bass as bass
import concourse.tile as tile
from concourse import mybir
from concourse.bass_utils import run_bass_kernel_spmd

BF16 = ml_dtypes.bfloat16
F32 = mybir.dt.float32
BF = mybir.dt.bfloat16

B, S, E, Q, H = 8, 4096, 1024, 4, 4
D = E // H          # 256
QH = H * Q          # 16
CH = E // 128       # 8 chunks of the feature dim
TILES = S // 128    # 32 s-tiles per core
SCALE = 1.0 / np.sqrt(D)
N_CORES = 8


def _patch_tile_drain():
    """The walrus build here caps sync-waits per instruction (1 for most
    opcodes, 2 for EventSemaphore), but the Tile scheduler freely attaches
    several. Split the excess onto preceding same-engine nops at commit
    time (commit order == final program order), and likewise for the exit
    drain's accumulated waits."""
    from concourse.tile import ScopedClock, TileContext

    orig_commit = TileContext._commit_instruction

    def _commit_instruction(self, inst, lazy_reg_writes=True):
        si = getattr(inst, "sync_info", None)
        cap = 2 if isinstance(inst, mybir.InstEventSemaphore) else 1
        engine = getattr(inst, "engine", None)
        if si is not None and si.on_wait and len(si.on_wait) > cap and engine:
            waits = list(si.on_wait)
            for w in waits[:-cap]:
                nop = mybir.InstNoOp(
                    name=self.nc.get_next_instruction_name(),
                    engine=engine,
                    bass_nofuse=True,
                    sync_info=mybir.SyncInfo(on_wait=[w], on_update=[]),
                    ins=[],
                    outs=[],
                )
                orig_commit(self, nop, lazy_reg_writes)
            inst.sync_info = mybir.SyncInfo(
                on_wait=waits[-cap:], on_update=list(si.on_update)
            )
        return orig_commit(self, inst, lazy_reg_writes)

    TileContext._commit_instruction = _commit_instruction

    def _drain_and_barrier(self, tick_clock, wait_clock):
        nc = self.nc
        drain_inst = nc.sync.drain()
        wait_clock.add_sem_waits(
            drain_inst.ins, ScopedClock({None: tick_clock.global_clock})
        )
        waits = list(drain_inst.ins.sync_info.on_wait)
        if len(waits) > 1:
            drain_inst.ins.sync_info.on_wait = [waits[0]]
            for w in waits[1:]:
                n = nc.sync.nop(nofuse=True)
                n.ins.sync_info = mybir.SyncInfo(on_wait=[w], on_update=[])

        nc.all_engine_barrier()
        assert self.sems is not None
        popped = nc._tile_sem_poison_stack.pop()
        assert popped is self._sem_poison
        nc.clear_and_free_semaphores(list(self.sems.allocated().values()))
        nc.all_engine_barrier()

    TileContext._drain_and_barrier = _drain_and_barrier


_patch_tile_drain()


def _patch_walrus_ldw_opt():
    """bir_verify_and_optimise hardcodes --enable-ldw-opt=false; enable it
    so LDWEIGHTS can use the fast-weight-load path / pull-ahead."""
    import os
    import concourse.bass_utils as bu

    # walrus rejects Tile's explicit InstLdweights with the opt enabled;
    # kept only as an experiment flag.
    if os.environ.get("BASS_LDW_OPT", "0") != "1":
        return
    orig_run = bu.run_command

    def run_command(argv, **kw):
        argv = [
            a.replace("--enable-ldw-opt=false", "--enable-ldw-opt=true")
            if isinstance(a, str)
            else a
            for a in argv
        ]
        return orig_run(argv, **kw)

    bu.run_command = run_command


_patch_walrus_ldw_opt()


def _build_nc() -> bass.Bass:
    nc = bass.Bass()

    hs = nc.declare_dram_parameter("hs", [S, E], F32, isOutput=False)
    qT = nc.declare_dram_parameter("qT", [128, CH * Q], BF, isOutput=False)
    wq = nc.declare_dram_parameter("wq", [E, E], BF, isOutput=False)   # Wq^T
    wk = nc.declare_dram_parameter("wk", [E, E], BF, isOutput=False)   # Wk natural
    wv = nc.declare_dram_parameter("wv", [E, E], BF, isOutput=False)   # Wv^T
    ow = nc.declare_dram_parameter("ow", [E, E], BF, isOutput=False)   # out_w^T
    bq = nc.declare_dram_parameter("bq", [1, E], BF, isOutput=False)
    bv = nc.declare_dram_parameter("bv", [1, E], BF, isOutput=False)
    ob = nc.declare_dram_parameter("ob", [1, E], BF, isOutput=False)
    idb = nc.declare_dram_parameter("idb", [128, 128], BF, isOutput=False)
    ones14 = nc.declare_dram_parameter("ones14", [1, 4], BF, isOutput=False)
    one11 = nc.declare_dram_parameter("one11", [1, 1], BF, isOutput=False)
    maskG = nc.declare_dram_parameter("maskG", [QH, H], BF, isOutput=False)
    maskA = nc.declare_dram_parameter("maskA", [QH, Q], BF, isOutput=False)

    pooled_out = nc.declare_dram_parameter("pooled", [1, E], F32, isOutput=True)
    attnw_out = nc.declare_dram_parameter("attn_w", [Q, S], F32, isOutput=True)

    Exp = mybir.ActivationFunctionType.Exp

    with tile.TileContext(nc) as tc:
        with (
            tc.tile_pool(name="consts", bufs=1) as consts,
            tc.tile_pool(name="weights", bufs=1) as weights,
            tc.tile_pool(name="small", bufs=1) as small,
        ):
            # ---- constants ----
            # tiny SWDGE warmer: absorbs the ~13us Q7/SWDGE spin-up so the
            # first real hs supertile DMA streams immediately
            warm_sb = consts.tile([128, 16], F32)
            nc.gpsimd.dma_start(warm_sb[:], hs[0:128, 0:16])

            idb_sb = consts.tile([128, 128], BF)
            nc.sync.dma_start(idb_sb[:], idb[:])
            ones14_sb = consts.tile([1, 4], BF)
            nc.sync.dma_start(ones14_sb[:], ones14[:])
            one11_sb = consts.tile([1, 1], BF)
            nc.sync.dma_start(one11_sb[:], one11[:])
            maskG_sb = consts.tile([QH, H], BF)
            nc.sync.dma_start(maskG_sb[:], maskG[:])
            maskA_sb = consts.tile([QH, Q], BF)
            nc.sync.dma_start(maskA_sb[:], maskA[:])
            bq_sb = consts.tile([1, E], BF)
            nc.sync.dma_start(bq_sb[:], bq[:])
            bv_sb = consts.tile([1, E], BF)
            nc.sync.dma_start(bv_sb[:], bv[:])
            ob_sb = consts.tile([1, E], BF)
            nc.sync.dma_start(ob_sb[:], ob[:])
            # qT arrives host-packed as [128, CH*Q] (single clean DMA)
            qT_sb = consts.tile([128, CH * Q], BF)
            nc.sync.dma_start(qT_sb[:], qT[:])

            # ---- prologue weights (needed before the scores pass) ----
            # split across both HWDGE rings; the SWDGE queue is reserved for
            # the hs stream so it can start immediately
            wq_sb = weights.tile([128, CH * E], BF)
            wk_sb = weights.tile([128, CH * E], BF)
            for dst, src in ((wq_sb, wq), (wk_sb, wk)):
                nc.scalar.dma_start(
                    dst[:].rearrange("p (c f) -> p c f", c=CH),
                    src[:].rearrange("(c p) f -> p c f", p=128),
                )

            # ---- prologue outputs (built inside the main pool scope) ----
            wt_sb = small.tile([128, CH * QH], BF)    # Wtilde chunks
            qhat_sb = small.tile([128, CH * QH], BF)  # Qhat chunks
            q_sb = small.tile([Q, E], BF)
            wtT_sb = small.tile([QH, E], BF)

            # ---- main streaming pass over hidden_states ----
            # SWDGE cast-DMA loads 4-tile supertiles f32->bf16 directly.
            # Per supertile g (512 rows of hs):
            #   transpose the 32 [128,128] blocks on PE into T (feature-major)
            #   scores2[16, 512] = sum_c wt_c^T @ T_c   (cheap 16-col ldweights)
            #   p2[:, g*512:+512] = exp(scores2), den partial via accum_out
            #   per 128-tile: p_tile[128,16] = transpose(p2 slice); num += p^T hs
            aw_sb = small.tile([Q, S], F32)
            p2_sb = small.tile([QH, S], BF)
            num_sb = small.tile([QH, E], BF)
            den8 = small.tile([QH, 2 * (NSUP := TILES // 4)], F32)
            den_sb = small.tile([QH, 1], F32)
            SUP = 4          # tiles per supertile DMA

            with (
                tc.tile_pool(name="stage", bufs=6) as stage,
                tc.tile_pool(name="tpool", bufs=4) as tpool,
                tc.tile_pool(name="ppool", bufs=6) as ppool,
                tc.tile_pool(name="psT", bufs=2, space="PSUM") as psT,
                tc.tile_pool(name="pscore", bufs=2, space="PSUM") as pscore,
                tc.tile_pool(name="psp", bufs=2, space="PSUM") as psp,
                tc.tile_pool(name="psnd", bufs=1, space="PSUM") as psnd,
            ):
                nd_ps = psnd.tile([QH, E], F32)

                def load_and_transpose(g):
                    # supertile 0 comes over the SP HWDGE ring as f32 (+DVE
                    # cast): HWDGE spins up in <1us while the SWDGE path,
                    # which carries the rest of the stream, takes ~13us to
                    # produce its first bytes.
                    r0 = g * SUP * 128
                    hs_b = stage.tile([128, SUP * E], BF, name="hs_b")
                    # per-tile DMAs: transposes gate on single 512KB tiles
                    for k in range(SUP):
                        nc.gpsimd.dma_start(
                            hs_b[:, k * E : (k + 1) * E],
                            hs[r0 + k * 128 : r0 + (k + 1) * 128, :],
                        )
                    # transposed supertile, laid out [128e, (c, 512s)]
                    T_sb = tpool.tile([128, CH, SUP * 128], BF, name="T_sb")
                    for k in range(SUP):
                        hv = hs_b[:, k * E : (k + 1) * E]
                        for half in range(2):
                            T_ps = psT.tile([128, 512], F32, name="T_ps")
                            for cc in range(4):
                                c = half * 4 + cc
                                nc.tensor.matmul(
                                    T_ps[:, cc * 128 : (cc + 1) * 128],
                                    lhsT=hv[:, c * 128 : (c + 1) * 128],
                                    rhs=idb_sb[:],
                                    start=True,
                                    stop=True,
                                )
                            eng = (
                                nc.scalar.copy
                                if (2 * k + half) % 2 == 0
                                else nc.vector.tensor_copy
                            )
                            eng(
                                T_sb[
                                    :, half * 4 : (half + 1) * 4, k * 128 : (k + 1) * 128
                                ],
                                T_ps[:].rearrange("p (c s) -> p c s", c=4),
                            )
                    return hs_b, T_sb

                def score_and_accumulate(g, hs_b, T_sb):
                    sc_ps = pscore.tile([QH, SUP * 128], F32, name="sc_ps")
                    for c in range(CH):
                        nc.tensor.matmul(
                            sc_ps[:],
                            lhsT=wt_sb[:, c * QH : (c + 1) * QH],
                            rhs=T_sb[:, c, :],
                            start=(c == 0),
                            stop=(c == CH - 1),
                        )
                    for hf in range(2):
                        nc.scalar.activation(
                            p2_sb[
                                :,
                                g * SUP * 128 + hf * 256 : g * SUP * 128
                                + (hf + 1) * 256,
                            ],
                            sc_ps[:, hf * 256 : (hf + 1) * 256],
                            Exp,
                            accum_out=den8[:, 2 * g + hf : 2 * g + hf + 1],
                        )

                    for k in range(SUP):
                        i = g * SUP + k
                        pt_ps = psp.tile([128, QH], F32, name="pt_ps")
                        nc.tensor.matmul(
                            pt_ps[:],
                            lhsT=p2_sb[:, i * 128 : (i + 1) * 128],
                            rhs=idb_sb[0:QH, 0:QH],
                            start=True,
                            stop=True,
                        )
                        p_t = ppool.tile([128, QH], BF, name="p_t")
                        nc.vector.tensor_copy(p_t[:], pt_ps[:])
                        for j in range(2):
                            nc.tensor.matmul(
                                nd_ps[0:QH, j * 512 : (j + 1) * 512],
                                lhsT=p_t[:],
                                rhs=hs_b[:, k * E + j * 512 : k * E + (j + 1) * 512],
                                start=(i == 0),
                                stop=(i == TILES - 1),
                            )

                def build_wtilde():
                    # q = queries@Wq^T + bq  (psum slots borrowed from the
                    # main-loop pools via tags so banks stay within budget)
                    q_h = [
                        pscore.tile([Q, 512], F32, name="q_h", tag="sc_ps")
                        for _ in range(2)
                    ]
                    for j in range(2):
                        for c in range(CH):
                            nc.tensor.matmul(
                                q_h[j][:],
                                lhsT=qT_sb[:, c * Q : (c + 1) * Q],
                                rhs=wq_sb[:, c * E + j * 512 : c * E + (j + 1) * 512],
                                start=(c == 0),
                                stop=False,
                            )
                        nc.tensor.matmul(
                            q_h[j][:],
                            lhsT=ones14_sb[:],
                            rhs=bq_sb[0:1, j * 512 : (j + 1) * 512],
                            start=False,
                            stop=True,
                        )
                        nc.scalar.copy(q_sb[:, j * 512 : (j + 1) * 512], q_h[j][:])

                    qt_ps = psT.tile([128, CH * Q], F32, name="qt_ps", tag="T_ps")
                    for r in range(CH):
                        nc.tensor.matmul(
                            qt_ps[:, r * Q : (r + 1) * Q],
                            lhsT=q_sb[:, r * 128 : (r + 1) * 128],
                            rhs=idb_sb[0:Q, 0:Q],
                            start=True,
                            stop=True,
                        )
                    nc.vector.memset(qhat_sb[:], 0.0)
                    for r in range(CH):
                        h = r // 2
                        nc.scalar.mul(
                            qhat_sb[:, r * QH + h * Q : r * QH + (h + 1) * Q],
                            qt_ps[:, r * Q : (r + 1) * Q],
                            SCALE,
                        )

                    # WtildeT[qh, e] = sum_g qhat[g, qh] * wk[g, e], then 8
                    # small transposes into the [e-major] wt chunks
                    for j in range(2):
                        wtT_ps = pscore.tile(
                            [QH, 512], F32, name="wtT_ps", tag="sc_ps"
                        )
                        for r in range(CH):
                            nc.tensor.matmul(
                                wtT_ps[:],
                                lhsT=qhat_sb[:, r * QH : (r + 1) * QH],
                                rhs=wk_sb[:, r * E + j * 512 : r * E + (j + 1) * 512],
                                start=(r == 0),
                                stop=(r == CH - 1),
                            )
                        nc.scalar.copy(
                            wtT_sb[:, j * 512 : (j + 1) * 512], wtT_ps[:]
                        )
                    for c in range(CH):
                        wt_ps = psp.tile([128, QH], F32, name="wt_ps", tag="pt_ps")
                        nc.tensor.matmul(
                            wt_ps[:],
                            lhsT=wtT_sb[:, c * 128 : (c + 1) * 128],
                            rhs=idb_sb[0:QH, 0:QH],
                            start=True,
                            stop=True,
                        )
                        nc.scalar.copy(wt_sb[:, c * QH : (c + 1) * QH], wt_ps[:])

                # software-pipelined; the first two supertiles' transposes are
                # traced BEFORE the prologue so the in-order PE stream starts
                # on work that only needs the (early) hs DMA, not the weights
                pending = [load_and_transpose(0), load_and_transpose(1)]
                build_wtilde()
                for g in range(2, NSUP):
                    pending.append(load_and_transpose(g))
                    score_and_accumulate(g - 2, *pending[g - 2])
                score_and_accumulate(NSUP - 2, *pending[NSUP - 2])
                score_and_accumulate(NSUP - 1, *pending[NSUP - 1])

                nc.scalar.copy(num_sb[:], nd_ps[0:QH, :])
                nc.vector.reduce_sum(den_sb[:], den8[:], axis=mybir.AxisListType.X)

            # ---- epilogue weights: traced late so they fill DMA gaps ----
            wv_sb = weights.tile([128, CH * E], BF)
            ow_sb = weights.tile([128, CH * E], BF)
            nc.scalar.dma_start(
                wv_sb[:].rearrange("p (c f) -> p c f", c=CH),
                wv[:].rearrange("(c p) f -> p c f", p=128),
            )
            nc.scalar.dma_start(
                ow_sb[:].rearrange("p (c f) -> p c f", c=CH),
                ow[:].rearrange("(c p) f -> p c f", p=128),
            )

            # ---- epilogue ----
            with (
                tc.tile_pool(name="awp", bufs=2, space="PSUM") as aw_pool,
            ):
                inv_den = small.tile([QH, 1], F32)
                nc.vector.reciprocal(inv_den[:], den_sb[:])
                invq = small.tile([QH, 1], F32)
                nc.vector.tensor_scalar_mul(invq[:], inv_den[:], 1.0 / Q)
                G_sb = small.tile([QH, H], BF)
                nc.vector.tensor_scalar_mul(G_sb[:], maskG_sb[:], invq[:])
                cA_sb = small.tile([QH, Q], BF)
                nc.vector.tensor_scalar_mul(cA_sb[:], maskA_sb[:], invq[:])

                for g in range(8):
                    awp = aw_pool.tile([Q, 512], F32, name="awp")
                    nc.tensor.matmul(
                        awp[:],
                        lhsT=cA_sb[:],
                        rhs=p2_sb[:, g * 512 : (g + 1) * 512],
                        start=True,
                        stop=True,
                    )
                    nc.scalar.copy(aw_sb[:, g * 512 : (g + 1) * 512], awp[:])

            with (
                tc.tile_pool(name="nbt", bufs=1, space="PSUM") as nbt_pool,
                tc.tile_pool(name="vbp", bufs=1, space="PSUM") as vb_pool,
                tc.tile_pool(name="cbt", bufs=1, space="PSUM") as cbt_pool,
                tc.tile_pool(name="plp", bufs=1, space="PSUM") as pl_pool,
            ):
                nbt_ps = nbt_pool.tile([128, CH * H], F32)
                for c in range(CH):
                    nc.tensor.matmul(
                        nbt_ps[:, c * H : (c + 1) * H],
                        lhsT=num_sb[:, c * 128 : (c + 1) * 128],
                        rhs=G_sb[:],
                        start=True,
                        stop=True,
                    )
                nbt_sb = small.tile([128, CH * H], BF)
                nc.scalar.copy(nbt_sb[:], nbt_ps[:])

                # cbar[0, h*D+d] = sum_e nbarT[e,h] * WvT[e, h*D+d] + bv[h*D+d]
                # computed per-head directly into partition 0 (engines cannot
                # read PSUM at unaligned base partitions).
                cb_ps = vb_pool.tile([1, E], F32)
                for h in range(H):
                    for c in range(CH):
                        nc.tensor.matmul(
                            cb_ps[0:1, h * D : (h + 1) * D],
                            lhsT=nbt_sb[:, c * H + h : c * H + h + 1],
                            rhs=wv_sb[:, c * E + h * D : c * E + (h + 1) * D],
                            start=(c == 0),
                            stop=False,
                        )
                    nc.tensor.matmul(
                        cb_ps[0:1, h * D : (h + 1) * D],
                        lhsT=one11_sb[:],
                        rhs=bv_sb[0:1, h * D : (h + 1) * D],
                        start=False,
                        stop=True,
                    )
                cbar_sb = small.tile([1, E], BF)
                nc.scalar.copy(cbar_sb[:], cb_ps[:])

                cbt_ps = cbt_pool.tile([128, CH], F32)
                for c in range(CH):
                    nc.tensor.matmul(
                        cbt_ps[:, c : c + 1],
                        lhsT=cbar_sb[0:1, c * 128 : (c + 1) * 128],
                        rhs=one11_sb[:],
                        start=True,
                        stop=True,
                    )
                cbt_sb = small.tile([128, CH], BF)
                nc.scalar.copy(cbt_sb[:], cbt_ps[:])

                pl_ps = pl_pool.tile([1, E], F32)
                for j in range(2):
                    for c in range(CH):
                        nc.tensor.matmul(
                            pl_ps[0:1, j * 512 : (j + 1) * 512],
                            lhsT=cbt_sb[:, c : c + 1],
                            rhs=ow_sb[:, c * E + j * 512 : c * E + (j + 1) * 512],
                            start=(c == 0),
                            stop=False,
                        )
                    nc.tensor.matmul(
                        pl_ps[0:1, j * 512 : (j + 1) * 512],
                        lhsT=one11_sb[:],
                        rhs=ob_sb[0:1, j * 512 : (j + 1) * 512],
                        start=False,
                        stop=True,
                    )
                pooled_sb = small.tile([1, E], F32)
                nc.scalar.copy(pooled_sb[:], pl_ps[:])

            nc.sync.dma_start(attnw_out[:], aw_sb[:])
            nc.sync.dma_start(pooled_out[:], pooled_sb[:])

    return nc


_NC_CACHE = None


def _get_nc():
    global _NC_CACHE
    if _NC_CACHE is None:
        _NC_CACHE = _build_nc()
    return _NC_CACHE


def _host_prep(hidden_states, queries, in_proj_w, in_proj_b, out_w):
    f32 = np.float32
    hidden_states = np.asarray(hidden_states, dtype=f32)
    queries = np.asarray(queries, dtype=f32)
    in_proj_w = np.asarray(in_proj_w, dtype=f32)
    in_proj_b = np.asarray(in_proj_b, dtype=f32)
    out_w = np.asarray(out_w, dtype=f32)

    maskG = np.zeros((QH, H), dtype=BF16)
    maskA = np.zeros((QH, Q), dtype=BF16)
    for h in range(H):
        for q in range(Q):
            maskG[h * Q + q, h] = 1
            maskA[h * Q + q, q] = 1

    # qT packed to the SBUF layout [128, (chunk, q)]
    qT_packed = np.ascontiguousarray(
        queries[0].T.reshape(CH, 128, Q).transpose(1, 0, 2).reshape(128, CH * Q)
    ).astype(BF16)

    shared = {
        "qT": qT_packed,
        "wq": np.ascontiguousarray(in_proj_w[0:E].T).astype(BF16),
        "wk": np.ascontiguousarray(in_proj_w[E : 2 * E]).astype(BF16),
        "wv": np.ascontiguousarray(in_proj_w[2 * E : 3 * E].T).astype(BF16),
        "ow": np.ascontiguousarray(out_w.T).astype(BF16),
        "bq": in_proj_b[0:E].reshape(1, E).astype(BF16),
        "bv": in_proj_b[2 * E : 3 * E].reshape(1, E).astype(BF16),
        "ob": None,  # filled by caller (out_b)
        "idb": np.eye(128, dtype=BF16),
        "ones14": np.ones((1, 4), dtype=BF16),
        "one11": np.ones((1, 1), dtype=BF16),
        "maskG": maskG,
        "maskA": maskA,
    }
    return hidden_states, shared


def kernel(hidden_states, queries, in_proj_w, in_proj_b, out_w, out_b, _trace=False):
    hidden_states, shared = _host_prep(
        hidden_states, queries, in_proj_w, in_proj_b, out_w
    )
    shared["ob"] = np.asarray(out_b, dtype=np.float32).reshape(1, E).astype(BF16)

    in_maps = []
    for b in range(N_CORES):
        m = dict(shared)
        m["hs"] = np.ascontiguousarray(hidden_states[b])
        in_maps.append(m)

    nc = _get_nc()
    res = run_bass_kernel_spmd(nc, in_maps, core_ids=list(range(N_CORES)), trace=_trace)

    pooled = np.stack([res.results[b]["pooled"][0] for b in range(N_CORES)])
    attn_w = np.stack([res.results[b]["attn_w"] for b in range(N_CORES)])
    if _trace:
        return (pooled, attn_w), res
    return pooled, attn_w
